# revision 1
# baseline (speedup 1.0000x reference)
"""Trainium2 Bass kernel for nn_Desc_Seq2Seq (2-layer LSTM encoder-decoder).

Self-contained: builds the Bass program, shards the batch 8-ways across
NeuronCores (data-parallel), runs via run_bass_kernel_spmd, gathers output.
"""
"""Bass/Tile kernel builder for the Seq2Seq 2-layer LSTM (encoder+decoder).

Per-core (B_local=64, 8 cores data-parallel):
  - Gates in PSUM [128, 512] per gate-chunk: partitions 0-63 = layer1,
    64-127 = layer0.  Gate order along 2048 reordered to (g, i, f, o).
  - Weights are the MOVING matmul operand (f32r), pre-transposed host-side.
  - Hidden states transposed in hT [128, 4*128] f32r: chunk k cols
    [128k,128k+64) = h1T, [128k+64,128k+128) = h0T.
  - c_all [128, 512] fp32: partitions 0-63 = c1, 64-127 = c0.
  - Wavefront tick t = layer1(step t) + layer0(step t+1).
"""
from contextlib import ExitStack
import numpy as np
import concourse.bass as bass
import concourse.bacc as bacc
import concourse.tile as tile
from concourse import mybir
from concourse.bass import DynSlice

F32 = mybir.dt.float32
F32R = mybir.dt.float32r
F16 = mybir.dt.float16
AF = mybir.ActivationFunctionType
OP = mybir.AluOpType

H = 512
G = 2048
NCH = 4
KCH = 4
BL = 64
FEAT = 128


def build_kernel(T_enc, pred_len, fc_b_val, enc_unroll=0, dec_unroll=0):
    nc = bacc.Bacc("TRN2", target_bir_lowering=False, debug=False,
                   num_devices=8)

    xT_d = nc.dram_tensor("xT", [T_enc, FEAT, BL], F16, kind="ExternalInput")
    wenc_d = nc.dram_tensor("wenc", [FEAT, 13 * G], F16, kind="ExternalInput")
    wdec_d = nc.dram_tensor("wdec", [FEAT, 12 * G], F16, kind="ExternalInput")
    benc1_d = nc.dram_tensor("benc1", [BL, G], F32, kind="ExternalInput")
    benc0_d = nc.dram_tensor("benc0", [BL, G], F32, kind="ExternalInput")
    bdec1_d = nc.dram_tensor("bdec1", [BL, G], F32, kind="ExternalInput")
    bdec0_d = nc.dram_tensor("bdec0", [BL, G], F32, kind="ExternalInput")
    ident_d = nc.dram_tensor("ident", [128, 128], F32, kind="ExternalInput")
    wdi0_d = nc.dram_tensor("wdi0", [1, G], F16, kind="ExternalInput")
    fcw_d = nc.dram_tensor("fcw", [FEAT, KCH], F16, kind="ExternalInput")
    ytf_d = nc.dram_tensor("ytf", [BL, pred_len], F32, kind="ExternalInput")
    tf1m_d = nc.dram_tensor("tf1m", [BL, pred_len], F32, kind="ExternalInput")
    xdec_d = nc.dram_tensor("xdec", [BL, 96 * 8], F32, kind="ExternalInput")
    out_d = nc.dram_tensor("out", [BL, pred_len], F32, kind="ExternalOutput")

    with ExitStack() as ctx:
        tc = ctx.enter_context(tile.TileContext(nc))
        state = ctx.enter_context(tc.tile_pool(name="state", bufs=1))
        psg = ctx.enter_context(tc.tile_pool(name="psg", bufs=6, space="PSUM"))
        psh = ctx.enter_context(tc.tile_pool(name="psh", bufs=2, space="PSUM"))
        ew = ctx.enter_context(tc.tile_pool(name="ew", bufs=2))
        xp = ctx.enter_context(tc.tile_pool(name="xp", bufs=4))

        hT = state.tile([128, KCH * 128], F16)
        c_all = state.tile([128, H], F32)
        ident = state.tile([128, 128], F32)
        b_all = state.tile([128, G], F32)
        wdi0 = state.tile([1, G], F16)
        fcw = state.tile([FEAT, KCH], F16)
        ytf = state.tile([BL, pred_len], F32)
        tf1m = state.tile([BL, pred_len], F32)
        outs = state.tile([BL, pred_len], F32)
        inpT = state.tile([1, BL], F16)

        nc.any.memset(hT[:], 0.0)
        nc.any.memset(c_all[:], 0.0)
        nc.sync.dma_start(ident[:], ident_d.ap())
        nc.sync.dma_start(b_all[0:BL, :], benc1_d.ap())
        nc.sync.dma_start(b_all[BL:128, :], benc0_d.ap())
        nc.sync.dma_start(wdi0[:], wdi0_d.ap())
        nc.sync.dma_start(fcw[:], fcw_d.ap())
        nc.sync.dma_start(ytf[:], ytf_d.ap())
        nc.sync.dma_start(tf1m[:], tf1m_d.ap())

        xT_ap = xT_d.ap()

        def wsl(wt, chunk, j):
            return wt[:, chunk * G + j * H: chunk * G + j * H + H]

        def h1T(k):
            return hT[:, 128 * k: 128 * k + BL]

        def h0T(k):
            return hT[:, 128 * k + BL: 128 * k + 128]

        def load_x(t_iv):
            xr = xp.tile([FEAT, BL], F16, tag="xr")
            nc.sync.dma_start(xr[:], xT_ap[DynSlice(t_iv, 1), :, :].squeeze(0))
            return xr

        def alloc_pg(tag="pg"):
            return [psg.tile([128, H], F32, tag="pg", name=f"pg{j}", bufs=6)
                    for j in range(NCH)]

        def emit_L0_mms(pg, we, E_IH0, E_HH0, xr):
            """encoder layer0(t+1) MMs -> pg[j][64:128] (col strips 2-3)"""
            for j in range(NCH):
                dst = pg[j][BL:128, :]
                nc.tensor.matmul(dst, xr[:], wsl(we, E_IH0, j), start=True,
                                 stop=False, tile_position=(0, BL),
                                 skip_group_check=True)
                for k in range(KCH):
                    nc.tensor.matmul(dst, h0T(k), wsl(we, E_HH0 + k, j),
                                     start=False, stop=(k == KCH - 1),
                                     tile_position=(0, BL),
                                     skip_group_check=True)

        def emit_dec_L0_mms(pg, wd, D_HH0):
            """decoder layer0(t+1) MMs -> pg[j][64:128]"""
            for j in range(NCH):
                dst = pg[j][BL:128, :]
                for k in range(KCH):
                    nc.tensor.matmul(dst, h0T(k), wsl(wd, D_HH0 + k, j),
                                     start=(k == 0), stop=False,
                                     tile_position=(0, BL),
                                     skip_group_check=True)
                nc.tensor.matmul(dst, inpT[:], wdi0[:, j * H:(j + 1) * H],
                                 start=False, stop=True, tile_position=(0, BL),
                                 skip_group_check=True)

        def emit_L1_mms(pg, wt, IH1, HH1):
            for j in range(NCH):
                dst = pg[j][0:BL, :]
                for k in range(KCH):
                    nc.tensor.matmul(dst, h0T(k), wsl(wt, IH1 + k, j),
                                     start=(k == 0), stop=False,
                                     tile_position=(0, 0),
                                     skip_group_check=True)
                for k in range(KCH):
                    nc.tensor.matmul(dst, h1T(k), wsl(wt, HH1 + k, j),
                                     start=False, stop=(k == KCH - 1),
                                     tile_position=(0, 0),
                                     skip_group_check=True)

        def emit_elementwise(pg, lo, hi):
            """bias+act+cell+h on partitions [lo:hi); transpose into hT.
            Per-gate ordering so the c-chain starts before o lands."""
            gs = ew.tile([128, G], F32, tag="gs", name="gs")
            act = ew.tile([128, G], F32, tag="act", name="act")
            ig = ew.tile([128, H], F32, tag="ig", name="ig")
            fc = ew.tile([128, H], F32, tag="fc", name="fc")
            tch = ew.tile([128, H], F32, tag="tch", name="tch")
            hnew = ew.tile([128, H], F32, tag="hnew", name="hnew")

            def gate(j, func):
                nc.vector.tensor_tensor(gs[lo:hi, j * H:(j + 1) * H],
                                        pg[j][lo:hi, :],
                                        b_all[lo:hi, j * H:(j + 1) * H],
                                        OP.add)
                nc.scalar.activation(act[lo:hi, j * H:(j + 1) * H],
                                     gs[lo:hi, j * H:(j + 1) * H], func)

            gate(0, AF.Tanh)      # g
            gate(1, AF.Sigmoid)   # i
            nc.vector.tensor_tensor(ig[lo:hi, :], act[lo:hi, H:2 * H],
                                    act[lo:hi, 0:H], OP.mult)
            gate(2, AF.Sigmoid)   # f
            nc.vector.tensor_tensor(fc[lo:hi, :], act[lo:hi, 2 * H:3 * H],
                                    c_all[lo:hi, :], OP.mult)
            nc.vector.tensor_tensor(c_all[lo:hi, :], ig[lo:hi, :],
                                    fc[lo:hi, :], OP.add)
            nc.scalar.activation(tch[lo:hi, :], c_all[lo:hi, :], AF.Tanh)
            gate(3, AF.Sigmoid)   # o
            nc.vector.tensor_tensor(hnew[lo:hi, :], act[lo:hi, 3 * H:4 * H],
                                    tch[lo:hi, :], OP.mult)
            for k in range(KCH):
                ph = psh.tile([128, 128], F32, tag="ph", name="ph")
                nc.tensor.transpose(ph[:, lo:hi],
                                    hnew[lo:hi, 128 * k:128 * k + 128],
                                    ident[lo:hi, lo:hi])
                nc.scalar.copy(hT[:, 128 * k + lo: 128 * k + hi],
                               ph[:, lo:hi])

        # ---------- encoder ----------
        with tc.tile_pool(name="wenc", bufs=1) as wenc_pool:
            we = wenc_pool.tile([FEAT, 13 * G], F16)
            nc.sync.dma_start(we[:], wenc_d.ap())
            E_IH0, E_HH0, E_IH1, E_HH1 = 0, 1, 5, 9

            pg0 = alloc_pg()
            emit_L0_mms(pg0, we, E_IH0, E_HH0, load_x(0))
            emit_elementwise(pg0, BL, 128)

            def enc_tick(iv):
                pg = alloc_pg()
                xr = load_x(iv + 1)
                emit_L1_mms(pg, we, E_IH1, E_HH1)
                emit_L0_mms(pg, we, E_IH0, E_HH0, xr)
                emit_elementwise(pg, 0, 128)

            if enc_unroll and T_enc > enc_unroll + 1:
                tc.For_i_unrolled(0, T_enc - 1, 1, enc_tick,
                                  max_unroll=enc_unroll)
            else:
                for t in range(T_enc - 1):
                    enc_tick(t)

            pgE = alloc_pg()
            emit_L1_mms(pgE, we, E_IH1, E_HH1)
            emit_elementwise(pgE, 0, BL)

        # ---------- decoder ----------
        nc.sync.dma_start(b_all[0:BL, :], bdec1_d.ap())
        nc.sync.dma_start(b_all[BL:128, :], bdec0_d.ap())
        wdec_pool = ctx.enter_context(tc.tile_pool(name="wdec", bufs=1))
        wd = wdec_pool.tile([FEAT, 12 * G], F16)
        nc.sync.dma_start(wd[:], wdec_d.ap())
        D_HH0, D_IH1, D_HH1 = 0, 4, 8

        xdec = state.tile([BL, 96 * 8], F32)
        nc.sync.dma_start(xdec[:], xdec_d.ap())
        dsum = ew.tile([BL, 1], F32, tag="dsum")
        nc.vector.tensor_reduce(dsum[:], xdec[:], axis=mybir.AxisListType.X, op=OP.add)
        pin = psh.tile([128, 128], F32, tag="ph", name="pin")
        nc.tensor.transpose(pin[0:1, 0:BL], dsum[:], ident[0:BL, 0:BL])
        nc.vector.tensor_copy(inpT[:], pin[0:1, 0:BL])

        def fc_and_select(t, last=False):
            """t may be a python int or a loop ScalarValue."""
            tsl = DynSlice(t, 1)
            po = psh.tile([128, 128], F32, tag="ph", name="po")
            for k in range(KCH):
                nc.tensor.matmul(po[0:BL, 0:1], h1T(k), fcw[:, k:k + 1],
                                 start=(k == 0), stop=(k == KCH - 1),
                                 tile_position=(0, 0), skip_group_check=True)
            nc.vector.tensor_scalar_add(outs[:, tsl], po[0:BL, 0:1],
                                        fc_b_val)
            if last:
                return
            # inp_next = (1-tf_t)*out + tf_t*y[t+1]   (ytf = tf*y_next)
            sel = ew.tile([BL, 1], F32, tag="sel")
            nc.vector.scalar_tensor_tensor(sel[:], outs[:, tsl],
                                           tf1m[:, tsl], ytf[:, tsl],
                                           op0=OP.mult, op1=OP.add)
            psel = psh.tile([128, 128], F32, tag="ph", name="psel")
            nc.tensor.transpose(psel[0:1, 0:BL], sel[:], ident[0:BL, 0:BL])
            nc.vector.tensor_copy(inpT[:], psel[0:1, 0:BL])

        pgD = alloc_pg()
        emit_dec_L0_mms(pgD, wd, D_HH0)
        emit_elementwise(pgD, BL, 128)

        def dec_tick(iv):
            pgT = alloc_pg()
            emit_L1_mms(pgT, wd, D_IH1, D_HH1)
            emit_elementwise(pgT, 0, BL)
            fc_and_select(iv)
            pgB = alloc_pg()
            emit_dec_L0_mms(pgB, wd, D_HH0)
            emit_elementwise(pgB, BL, 128)

        if dec_unroll and pred_len > dec_unroll + 1:
            tc.For_i_unrolled(0, pred_len - 1, 1, dec_tick,
                              max_unroll=dec_unroll)
        else:
            for t in range(pred_len - 1):
                dec_tick(t)

        pgF = alloc_pg()
        emit_L1_mms(pgF, wd, D_IH1, D_HH1)
        emit_elementwise(pgF, 0, BL)
        fc_and_select(pred_len - 1, last=True)

        nc.sync.dma_start(out_d.ap(), outs[:])

    nc.compile()
    return nc


# ---------------- host-side packing ----------------
GATE_ORDER = np.concatenate([np.arange(2 * H, 3 * H),
                             np.arange(0, H),
                             np.arange(H, 2 * H),
                             np.arange(3 * H, 4 * H)])


def pack_w(W):
    return np.ascontiguousarray(np.asarray(W)[GATE_ORDER, :].T
                                .astype(np.float16))


def pack_wenc(Wih0, Whh0, Wih1, Whh1):
    chunks = [pack_w(Wih0)]
    for Wt in (Whh0, Wih1, Whh1):
        t = pack_w(Wt)
        chunks += [np.ascontiguousarray(t[k * 128:(k + 1) * 128])
                   for k in range(4)]
    return np.ascontiguousarray(np.concatenate(chunks, axis=1))


def pack_wdec(Whh0, Wih1, Whh1):
    chunks = []
    for Wt in (Whh0, Wih1, Whh1):
        t = pack_w(Wt)
        chunks += [np.ascontiguousarray(t[k * 128:(k + 1) * 128])
                   for k in range(4)]
    return np.ascontiguousarray(np.concatenate(chunks, axis=1))


def pack_bias(b):
    return np.ascontiguousarray(
        np.broadcast_to(b[GATE_ORDER], (BL, G)).astype(np.float32))


def make_in_map(core, T_enc, pred_len, inp):
    sl = slice(core * BL, core * BL + BL)
    tf = ((np.asarray(inp["tf_mask"])[:pred_len] != 0)
          & (np.arange(pred_len) < pred_len - 1)).astype(np.float32)
    y_next = np.zeros((BL, pred_len), np.float32)
    y_next[:, :pred_len - 1] = np.asarray(inp["y"])[sl, 1:pred_len, 0]
    ytf = np.ascontiguousarray(y_next * tf[None, :])
    tf1m = np.ascontiguousarray(np.broadcast_to(1.0 - tf, (BL, pred_len))
                                .astype(np.float32))
    xT = np.ascontiguousarray(
        np.asarray(inp["X_encode"])[sl, :T_enc].transpose(1, 2, 0)
        .astype(np.float16))
    return {
        "xT": xT,
        "wenc": pack_wenc(inp["enc_W_ih0"], inp["enc_W_hh0"],
                          inp["enc_W_ih1"], inp["enc_W_hh1"]),
        "wdec": pack_wdec(inp["dec_W_hh0"], inp["dec_W_ih1"],
                          inp["dec_W_hh1"]),
        "benc1": pack_bias(np.asarray(inp["enc_b_ih1"]) + np.asarray(inp["enc_b_hh1"])),
        "benc0": pack_bias(np.asarray(inp["enc_b_ih0"]) + np.asarray(inp["enc_b_hh0"])),
        "bdec1": pack_bias(np.asarray(inp["dec_b_ih1"]) + np.asarray(inp["dec_b_hh1"])),
        "bdec0": pack_bias(np.asarray(inp["dec_b_ih0"]) + np.asarray(inp["dec_b_hh0"])),
        "ident": np.eye(128, dtype=np.float32),
        "wdi0": np.ascontiguousarray(
            np.asarray(inp["dec_W_ih0"])[GATE_ORDER, 0][None, :]
            .astype(np.float16)),
        "fcw": np.ascontiguousarray(
            np.asarray(inp["fc_W"])[0].reshape(4, 128).T.astype(np.float16)),
        "ytf": ytf, "tf1m": tf1m,
        "xdec": np.ascontiguousarray(
            np.asarray(inp["X_decode"])[sl].reshape(BL, -1)
            .astype(np.float32)),
    }


def unpack_out(results, pred_len):
    full = np.zeros((8 * BL, pred_len, 1), np.float32)
    for c in range(8):
        full[c * BL:(c + 1) * BL, :, 0] = results[c]["out"]
    return full


# ---------------- public entry point ----------------
_NC_CACHE = {}


def _get_nc(T_enc, pred_len, fc_b_val):
    key = (T_enc, pred_len, float(fc_b_val))
    if key not in _NC_CACHE:
        _NC_CACHE[key] = build_kernel(T_enc, pred_len, float(fc_b_val),
                                      enc_unroll=4, dec_unroll=4)
    return _NC_CACHE[key]


def kernel(**inputs):
    from concourse.bass_utils import run_bass_kernel_spmd
    inp = {k: np.asarray(v) for k, v in inputs.items()}
    B, T_enc, _ = inp["X_encode"].shape
    pred_len = inp["y"].shape[1]
    assert B == 8 * BL, f"expected batch {8*BL}, got {B}"
    nc = _get_nc(T_enc, pred_len, float(inp["fc_b"][0]))
    in_maps = [make_in_map(c, T_enc, pred_len, inp) for c in range(8)]
    res = run_bass_kernel_spmd(nc, in_maps, core_ids=list(range(8)))
    return unpack_out(res.results, pred_len).astype(np.float32)



# revision 2
# speedup vs baseline: 1.5653x; 1.5653x over previous
"""Trainium2 Bass kernel for nn_Desc_Seq2Seq (2-layer LSTM encoder-decoder).

Self-contained: builds the Bass program, shards the batch 8-ways across
NeuronCores (data-parallel), runs via run_bass_kernel_spmd, gathers output.

Pipeline-optimized v2:
  - Gate order (f, i, g, o): the c-chain (fc -> c -> tanh_c) overlaps the
    i/g/o matmul blocks instead of serializing after them.
  - Per-tick PE order: [x-mms] [per gate j: HH1 k0-3, IH1 k0-3, HH0 k0-3]
    [4 transposes].  x-mms at tick start cover the previous tick's o-tail.
  - o-gate add/act/hnew/transpose/copy split into 4x128-col pieces so the
    first hT piece lands ~1.3us after the last matmul.
  - hnew/transposes in fp16 (single PE pass vs fp32's double pass).
  - Decoder tick: [HH1 x16][IH1 x16][HH0 j01][L1-cell][fc][HH0 j23][inp]
    [L0-cell] so the fc/select chain hides under HH0 matmuls.
"""
from contextlib import ExitStack
import numpy as np
import concourse.bass as bass
import concourse.bacc as bacc
import concourse.tile as tile
from concourse import mybir
from concourse.bass import DynSlice

F32 = mybir.dt.float32
F32R = mybir.dt.float32r
F16 = mybir.dt.float16
AF = mybir.ActivationFunctionType
OP = mybir.AluOpType

H = 512
G = 2048
NCH = 4
KCH = 4
BL = 64
FEAT = 128


def build_kernel(T_enc, pred_len, fc_b_val, enc_unroll=0, dec_unroll=0):
    nc = bacc.Bacc("TRN2", target_bir_lowering=False, debug=False,
                   num_devices=8)

    xT_d = nc.dram_tensor("xT", [T_enc, FEAT, BL], F16, kind="ExternalInput")
    wenc_d = nc.dram_tensor("wenc", [FEAT, 13 * G], F16, kind="ExternalInput")
    wdec_d = nc.dram_tensor("wdec", [FEAT, 12 * G], F16, kind="ExternalInput")
    benc1_d = nc.dram_tensor("benc1", [BL, G], F32, kind="ExternalInput")
    benc0_d = nc.dram_tensor("benc0", [BL, G], F32, kind="ExternalInput")
    bdec1_d = nc.dram_tensor("bdec1", [BL, G], F32, kind="ExternalInput")
    bdec0_d = nc.dram_tensor("bdec0", [BL, G], F32, kind="ExternalInput")
    ident_d = nc.dram_tensor("ident", [128, 128], F32, kind="ExternalInput")
    identf_d = nc.dram_tensor("identf", [128, 128], F16, kind="ExternalInput")
    wdi0_d = nc.dram_tensor("wdi0", [1, G], F16, kind="ExternalInput")
    fcw_d = nc.dram_tensor("fcw", [FEAT, KCH], F16, kind="ExternalInput")
    ytf_d = nc.dram_tensor("ytf", [BL, pred_len], F32, kind="ExternalInput")
    tf1m_d = nc.dram_tensor("tf1m", [BL, pred_len], F32, kind="ExternalInput")
    xdec_d = nc.dram_tensor("xdec", [BL, 96 * 8], F32, kind="ExternalInput")
    out_d = nc.dram_tensor("out", [BL, pred_len], F32, kind="ExternalOutput")

    with ExitStack() as ctx:
        tc = ctx.enter_context(tile.TileContext(nc))
        state = ctx.enter_context(tc.tile_pool(name="state", bufs=1))
        psg = ctx.enter_context(tc.tile_pool(name="psg", bufs=6, space="PSUM"))
        psh = ctx.enter_context(tc.tile_pool(name="psh", bufs=2, space="PSUM"))
        ew = ctx.enter_context(tc.tile_pool(name="ew", bufs=2))
        xp = ctx.enter_context(tc.tile_pool(name="xp", bufs=4))

        hT = state.tile([128, KCH * 128], F16)
        c_all = state.tile([128, H], F32)
        ident = state.tile([128, 128], F32)
        identf = state.tile([128, 128], F16)
        b_all = state.tile([128, G], F32)
        wdi0 = state.tile([1, G], F16)
        fcw = state.tile([FEAT, KCH], F16)
        ytf = state.tile([BL, pred_len], F32)
        tf1m = state.tile([BL, pred_len], F32)
        outs = state.tile([BL, pred_len], F32)
        inpT = state.tile([1, BL], F16)

        nc.any.memset(hT[:], 0.0)
        nc.any.memset(c_all[:], 0.0)
        nc.sync.dma_start(ident[:], ident_d.ap())
        nc.sync.dma_start(identf[:], identf_d.ap())
        nc.sync.dma_start(b_all[0:BL, :], benc1_d.ap())
        nc.sync.dma_start(b_all[BL:128, :], benc0_d.ap())
        nc.sync.dma_start(wdi0[:], wdi0_d.ap())
        nc.sync.dma_start(fcw[:], fcw_d.ap())
        nc.sync.dma_start(ytf[:], ytf_d.ap())
        nc.sync.dma_start(tf1m[:], tf1m_d.ap())

        xT_ap = xT_d.ap()

        def wsl(wt, chunk, j):
            return wt[:, chunk * G + j * H: chunk * G + j * H + H]

        def h1T(k):
            return hT[:, 128 * k: 128 * k + BL]

        def h0T(k):
            return hT[:, 128 * k + BL: 128 * k + 128]

        def load_x(t_iv):
            xr = xp.tile([FEAT, BL], F16, tag="xr")
            nc.sync.dma_start(xr[:], xT_ap[DynSlice(t_iv, 1), :, :].squeeze(0))
            return xr

        def alloc_pg(tag="pg"):
            return [psg.tile([128, H], F32, tag="pg", name=f"pg{j}", bufs=6)
                    for j in range(NCH)]

        def mm1(pg, j, st, w, start, stop):
            nc.tensor.matmul(pg[j][0:BL, :], st, w, start=start, stop=stop,
                             tile_position=(0, 0), skip_group_check=True)

        def mm0(pg, j, st, w, start, stop):
            nc.tensor.matmul(pg[j][BL:128, :], st, w, start=start, stop=stop,
                             tile_position=(0, BL), skip_group_check=True)

        def emit_cell(pg, lo, hi):
            """LSTM cell elementwise for partitions [lo:hi).
            Gate chunks: j0=f(Sig) j1=i(Sig) j2=g(Tanh) j3=o(Sig).
            o-gate is piece-split (4x128 cols); hnew/transposes fp16."""
            gs = ew.tile([128, G], F32, tag="gs", name="gs")
            act = ew.tile([128, G], F32, tag="act", name="act")
            fcp = ew.tile([128, H], F32, tag="fcp", name="fcp")
            ig = ew.tile([128, H], F32, tag="ig", name="ig")
            tch = ew.tile([128, H], F32, tag="tch", name="tch")
            hnew = ew.tile([128, H], F16, tag="hnew", name="hnew")
            sl = slice(lo, hi)

            def gadd(j):
                nc.vector.tensor_tensor(gs[sl, j * H:(j + 1) * H],
                                        pg[j][sl, :],
                                        b_all[sl, j * H:(j + 1) * H], OP.add)

            def gact(j, func):
                nc.scalar.activation(act[sl, j * H:(j + 1) * H],
                                     gs[sl, j * H:(j + 1) * H], func)

            gadd(0)                      # f
            gact(0, AF.Sigmoid)
            nc.vector.tensor_tensor(fcp[sl, :], act[sl, 0:H], c_all[sl, :],
                                    OP.mult)
            gadd(1)                      # i
            gact(1, AF.Sigmoid)
            gadd(2)                      # g
            gact(2, AF.Tanh)
            nc.vector.tensor_tensor(ig[sl, :], act[sl, H:2 * H],
                                    act[sl, 2 * H:3 * H], OP.mult)
            for a in range(2):           # c + tanh(c) in 2x256 pieces
                cs = slice(a * 256, (a + 1) * 256)
                nc.vector.tensor_tensor(c_all[sl, cs], fcp[sl, cs],
                                        ig[sl, cs], OP.add)
                nc.scalar.activation(tch[sl, cs], c_all[sl, cs], AF.Tanh)
            # o-gate: per-128-piece add/act/hnew/transpose/copy
            for p in range(KCH):
                ps = slice(3 * H + p * 128, 3 * H + p * 128 + 128)
                hs = slice(p * 128, (p + 1) * 128)
                nc.vector.tensor_tensor(gs[sl, ps], pg[3][sl, hs],
                                        b_all[sl, ps], OP.add)
                nc.scalar.activation(act[sl, ps], gs[sl, ps], AF.Sigmoid)
                nc.vector.tensor_tensor(hnew[sl, hs], act[sl, ps],
                                        tch[sl, hs], OP.mult)
                ph = psh.tile([128, 128], F16, tag="ph", name="ph")
                nc.tensor.transpose(ph[:, sl], hnew[sl, hs], identf[sl, sl])
                nc.scalar.copy(hT[:, 128 * p + lo: 128 * p + hi], ph[:, sl])

        # ---------- encoder ----------
        with tc.tile_pool(name="wenc", bufs=1) as wenc_pool:
            we = wenc_pool.tile([FEAT, 13 * G], F16)
            nc.sync.dma_start(we[:], wenc_d.ap())
            E_IH0, E_HH0, E_IH1, E_HH1 = 0, 1, 5, 9

            # prologue: L0 step 0 (h0=0, c0=0 -> only the x projection)
            pg0 = alloc_pg()
            xr0 = load_x(0)
            for j in range(NCH):
                mm0(pg0, j, xr0[:], wsl(we, E_IH0, j), True, True)
            emit_cell(pg0, BL, 128)

            def enc_tick(iv):
                pg = alloc_pg()
                xr = load_x(iv + 1)
                # x-mms first: no hT dependency, covers previous o-tail
                for j in range(NCH):
                    mm0(pg, j, xr[:], wsl(we, E_IH0, j), True, False)
                for j in range(NCH):
                    for k in range(KCH):
                        mm1(pg, j, h1T(k), wsl(we, E_HH1 + k, j), k == 0,
                            False)
                    for k in range(KCH):
                        mm1(pg, j, h0T(k), wsl(we, E_IH1 + k, j), False,
                            k == KCH - 1)
                    for k in range(KCH):
                        mm0(pg, j, h0T(k), wsl(we, E_HH0 + k, j), False,
                            k == KCH - 1)
                emit_cell(pg, 0, 128)

            if enc_unroll and T_enc > enc_unroll + 1:
                tc.For_i_unrolled(0, T_enc - 1, 1, enc_tick,
                                  max_unroll=enc_unroll)
            else:
                for t in range(T_enc - 1):
                    enc_tick(t)

            # epilogue: L1 step T-1
            pgE = alloc_pg()
            for j in range(NCH):
                for k in range(KCH):
                    mm1(pgE, j, h1T(k), wsl(we, E_HH1 + k, j), k == 0, False)
                for k in range(KCH):
                    mm1(pgE, j, h0T(k), wsl(we, E_IH1 + k, j), False,
                        k == KCH - 1)
            emit_cell(pgE, 0, BL)

        # ---------- decoder ----------
        nc.sync.dma_start(b_all[0:BL, :], bdec1_d.ap())
        nc.sync.dma_start(b_all[BL:128, :], bdec0_d.ap())
        wdec_pool = ctx.enter_context(tc.tile_pool(name="wdec", bufs=1))
        wd = wdec_pool.tile([FEAT, 12 * G], F16)
        nc.sync.dma_start(wd[:], wdec_d.ap())
        D_HH0, D_IH1, D_HH1 = 0, 4, 8

        xdec = state.tile([BL, 96 * 8], F32)
        nc.sync.dma_start(xdec[:], xdec_d.ap())
        dsum = ew.tile([BL, 1], F32, tag="dsum")
        nc.vector.tensor_reduce(dsum[:], xdec[:], axis=mybir.AxisListType.X,
                                op=OP.add)
        pin = psh.tile([128, 128], F32, tag="ph", name="pin")
        nc.tensor.transpose(pin[0:1, 0:BL], dsum[:], ident[0:BL, 0:BL])
        nc.vector.tensor_copy(inpT[:], pin[0:1, 0:BL])

        def fc_and_select(t, last=False):
            """t may be a python int or a loop ScalarValue."""
            tsl = DynSlice(t, 1)
            po = psh.tile([128, 128], F32, tag="ph", name="po")
            for k in range(KCH):
                nc.tensor.matmul(po[0:BL, 0:1], h1T(k), fcw[:, k:k + 1],
                                 start=(k == 0), stop=(k == KCH - 1),
                                 tile_position=(0, 0), skip_group_check=True)
            nc.vector.tensor_scalar_add(outs[:, tsl], po[0:BL, 0:1],
                                        fc_b_val)
            if last:
                return
            # inp_next = (1-tf_t)*out + tf_t*y[t+1]   (ytf = tf*y_next)
            sel = ew.tile([BL, 1], F32, tag="sel")
            nc.vector.scalar_tensor_tensor(sel[:], outs[:, tsl],
                                           tf1m[:, tsl], ytf[:, tsl],
                                           op0=OP.mult, op1=OP.add)
            psel = psh.tile([128, 128], F32, tag="ph", name="psel")
            nc.tensor.transpose(psel[0:1, 0:BL], sel[:], ident[0:BL, 0:BL])
            nc.vector.tensor_copy(inpT[:], psel[0:1, 0:BL])

        # decoder prologue: L0 step 0 (uses enc-final h0)
        pgD = alloc_pg()
        for j in range(NCH):
            for k in range(KCH):
                mm0(pgD, j, h0T(k), wsl(wd, D_HH0 + k, j), k == 0, False)
        for j in range(NCH):
            mm0(pgD, j, inpT[:], wdi0[:, j * H:(j + 1) * H], False, True)
        emit_cell(pgD, BL, 128)

        def dec_tick(iv):
            pg = alloc_pg()
            # L1 matmuls: HH1 all first (h1T ready early), then IH1 (h0T)
            for j in range(NCH):
                for k in range(KCH):
                    mm1(pg, j, h1T(k), wsl(wd, D_HH1 + k, j), k == 0, False)
            for j in range(NCH):
                for k in range(KCH):
                    mm1(pg, j, h0T(k), wsl(wd, D_IH1 + k, j), False,
                        k == KCH - 1)
            # HH0 first half (covers L1 cell chain)
            for j in (0, 1):
                for k in range(KCH):
                    mm0(pg, j, h0T(k), wsl(wd, D_HH0 + k, j), k == 0, False)
            emit_cell(pg, 0, BL)        # L1 cell (+ its transposes on PE)
            fc_and_select(iv)
            # HH0 second half (covers fc/select chain)
            for j in (2, 3):
                for k in range(KCH):
                    mm0(pg, j, h0T(k), wsl(wd, D_HH0 + k, j), k == 0, False)
            for j in range(NCH):
                mm0(pg, j, inpT[:], wdi0[:, j * H:(j + 1) * H], False, True)
            emit_cell(pg, BL, 128)      # L0 cell

        if dec_unroll and pred_len > dec_unroll + 1:
            tc.For_i_unrolled(0, pred_len - 1, 1, dec_tick,
                              max_unroll=dec_unroll)
        else:
            for t in range(pred_len - 1):
                dec_tick(t)

        pgF = alloc_pg()
        for j in range(NCH):
            for k in range(KCH):
                mm1(pgF, j, h1T(k), wsl(wd, D_HH1 + k, j), k == 0, False)
            for k in range(KCH):
                mm1(pgF, j, h0T(k), wsl(wd, D_IH1 + k, j), False,
                    k == KCH - 1)
        emit_cell(pgF, 0, BL)
        fc_and_select(pred_len - 1, last=True)

        nc.sync.dma_start(out_d.ap(), outs[:])

    nc.compile()
    return nc


# ---------------- host-side packing ----------------
# packed gate order: (f, i, g, o); PyTorch rows are (i, f, g, o)
GATE_ORDER = np.concatenate([np.arange(H, 2 * H),
                             np.arange(0, H),
                             np.arange(2 * H, 3 * H),
                             np.arange(3 * H, 4 * H)])


def pack_w(W):
    return np.ascontiguousarray(np.asarray(W)[GATE_ORDER, :].T
                                .astype(np.float16))


def pack_wenc(Wih0, Whh0, Wih1, Whh1):
    chunks = [pack_w(Wih0)]
    for Wt in (Whh0, Wih1, Whh1):
        t = pack_w(Wt)
        chunks += [np.ascontiguousarray(t[k * 128:(k + 1) * 128])
                   for k in range(4)]
    return np.ascontiguousarray(np.concatenate(chunks, axis=1))


def pack_wdec(Whh0, Wih1, Whh1):
    chunks = []
    for Wt in (Whh0, Wih1, Whh1):
        t = pack_w(Wt)
        chunks += [np.ascontiguousarray(t[k * 128:(k + 1) * 128])
                   for k in range(4)]
    return np.ascontiguousarray(np.concatenate(chunks, axis=1))


def pack_bias(b):
    return np.ascontiguousarray(
        np.broadcast_to(b[GATE_ORDER], (BL, G)).astype(np.float32))


def make_in_map(core, T_enc, pred_len, inp):
    sl = slice(core * BL, core * BL + BL)
    tf = ((np.asarray(inp["tf_mask"])[:pred_len] != 0)
          & (np.arange(pred_len) < pred_len - 1)).astype(np.float32)
    y_next = np.zeros((BL, pred_len), np.float32)
    y_next[:, :pred_len - 1] = np.asarray(inp["y"])[sl, 1:pred_len, 0]
    ytf = np.ascontiguousarray(y_next * tf[None, :])
    tf1m = np.ascontiguousarray(np.broadcast_to(1.0 - tf, (BL, pred_len))
                                .astype(np.float32))
    xT = np.ascontiguousarray(
        np.asarray(inp["X_encode"])[sl, :T_enc].transpose(1, 2, 0)
        .astype(np.float16))
    return {
        "xT": xT,
        "wenc": pack_wenc(inp["enc_W_ih0"], inp["enc_W_hh0"],
                          inp["enc_W_ih1"], inp["enc_W_hh1"]),
        "wdec": pack_wdec(inp["dec_W_hh0"], inp["dec_W_ih1"],
                          inp["dec_W_hh1"]),
        "benc1": pack_bias(np.asarray(inp["enc_b_ih1"]) + np.asarray(inp["enc_b_hh1"])),
        "benc0": pack_bias(np.asarray(inp["enc_b_ih0"]) + np.asarray(inp["enc_b_hh0"])),
        "bdec1": pack_bias(np.asarray(inp["dec_b_ih1"]) + np.asarray(inp["dec_b_hh1"])),
        "bdec0": pack_bias(np.asarray(inp["dec_b_ih0"]) + np.asarray(inp["dec_b_hh0"])),
        "ident": np.eye(128, dtype=np.float32),
        "identf": np.eye(128, dtype=np.float16),
        "wdi0": np.ascontiguousarray(
            np.asarray(inp["dec_W_ih0"])[GATE_ORDER, 0][None, :]
            .astype(np.float16)),
        "fcw": np.ascontiguousarray(
            np.asarray(inp["fc_W"])[0].reshape(4, 128).T.astype(np.float16)),
        "ytf": ytf, "tf1m": tf1m,
        "xdec": np.ascontiguousarray(
            np.asarray(inp["X_decode"])[sl].reshape(BL, -1)
            .astype(np.float32)),
    }


def unpack_out(results, pred_len):
    full = np.zeros((8 * BL, pred_len, 1), np.float32)
    for c in range(8):
        full[c * BL:(c + 1) * BL, :, 0] = results[c]["out"]
    return full


# ---------------- public entry point ----------------
_NC_CACHE = {}


def _get_nc(T_enc, pred_len, fc_b_val):
    key = (T_enc, pred_len, float(fc_b_val))
    if key not in _NC_CACHE:
        _NC_CACHE[key] = build_kernel(T_enc, pred_len, float(fc_b_val),
                                      enc_unroll=8, dec_unroll=8)
    return _NC_CACHE[key]


def kernel(**inputs):
    from concourse.bass_utils import run_bass_kernel_spmd
    inp = {k: np.asarray(v) for k, v in inputs.items()}
    B, T_enc, _ = inp["X_encode"].shape
    pred_len = inp["y"].shape[1]
    assert B == 8 * BL, f"expected batch {8*BL}, got {B}"
    nc = _get_nc(T_enc, pred_len, float(inp["fc_b"][0]))
    in_maps = [make_in_map(c, T_enc, pred_len, inp) for c in range(8)]
    res = run_bass_kernel_spmd(nc, in_maps, core_ids=list(range(8)))
    return unpack_out(res.results, pred_len).astype(np.float32)
